# revision 2
# baseline (speedup 1.0000x reference)
"""AttentionDownSample Trainium2 kernel v5 (8 NeuronCores, data-parallel over batch).

vs v4:
  - broadcast PSUM restructured into four 1-bank quarter tiles (dy,h),
    pool bufs=3 -> quarter-granular PE->scalar evac pipelining instead of
    whole-chunk turnaround.
  - psum_q double-buffered (bank budget: pk*2 + pq*2 + pwt*1 + pu*3 = 8).
  - pipeline deepened to 4 stages: b(ch-3), a2(ch-2), a1(ch), oldest
    work first in every engine FIFO.
"""

import os
import sys

sys.path.insert(0, "/opt/trn_rl_repo")

import numpy as np

B, C, H, W = 8, 128, 256, 256
CR = 32
NH, NW = H // 2, W // 2
N_CORES = 8
R = 4                      # output rows per chunk
N_CHUNKS = NH // R         # 32
SLAB_ROWS_LIST = [8, 24, 32, 32, 32, 32, 32, 32, 32]  # tapered head
N_SLABS = len(SLAB_ROWS_LIST)
SLAB_BASE = [sum(SLAB_ROWS_LIST[:i]) for i in range(N_SLABS)]
CHUNKS_PER_SLAB_LIST = [r // (2 * R) for r in SLAB_ROWS_LIST]
CHUNK_SLAB = []  # chunk -> (slab, local row base)
for _s, _n in enumerate(CHUNKS_PER_SLAB_LIST):
    for _c in range(_n):
        CHUNK_SLAB.append((_s, _c * 2 * R))
SLAB_FIRST_CHUNK = {}
for _ch, (_s, _lr) in enumerate(CHUNK_SLAB):
    if _s not in SLAB_FIRST_CHUNK:
        SLAB_FIRST_CHUNK[_s] = _ch


def build_bass():
    import concourse.bass as bass
    import concourse.mybir as mybir
    from concourse import bacc, tile

    f32 = mybir.dt.float32
    bf16 = mybir.dt.bfloat16
    nc = bacc.Bacc()

    fm = nc.declare_dram_parameter("fm", [C, H, W], f32, isOutput=False)
    wqk = nc.declare_dram_parameter("wqk", [C, 2 * CR], f32, isOutput=False)
    ident = nc.declare_dram_parameter("ident", [128, 128], f32, isOutput=False)
    sel = nc.declare_dram_parameter("sel", [4, 4, 128], f32, isOutput=False)
    out = nc.declare_dram_parameter("out", [C, NH, NW], bf16, isOutput=True)

    with tile.TileContext(nc) as tc:
        with (
            tc.tile_pool(name="const", bufs=1) as cpool,
            tc.tile_pool(name="slabs", bufs=1) as spool,
            tc.tile_pool(name="work", bufs=3) as wpool,
            tc.tile_pool(name="psA", bufs=2, space="PSUM") as ppoolA,
            tc.tile_pool(name="psB", bufs=1, space="PSUM") as ppoolB,
        ):
            wqk_s = cpool.tile([C, 2 * CR], bf16)
            nc.gpsimd.dma_start(wqk_s[:], wqk[:])
            ident_s = cpool.tile([128, 128], bf16)
            nc.gpsimd.dma_start(ident_s[:], ident[:])
            sel_s = cpool.tile([4, 4, 128], bf16)
            nc.gpsimd.dma_start(sel_s[:], sel[:])

            # even/odd-strided loads: 1KB descriptors stream at ~325GB/s.
            slabs = [None] * N_SLABS

            def load_slab(s):
                rows = SLAB_ROWS_LIST[s]
                sl = spool.tile([C, rows, W], bf16, tag=f"slab{s}")
                base = SLAB_BASE[s]
                nc.gpsimd.dma_start(
                    sl[:, 0:rows:2, :],
                    fm[:, base : base + rows : 2, :],
                )
                nc.gpsimd.dma_start(
                    sl[:, 1:rows:2, :],
                    fm[:, base + 1 : base + rows : 2, :],
                )
                slabs[s] = sl

            N_HEAD_SLABS = 3
            for s in range(N_HEAD_SLABS):
                load_slab(s)

            state = {}

            def stage_a1(ch):
                """Projections + softmax chain -> wgt[ch] (SBUF bf16)."""
                s, lr = CHUNK_SLAB[ch]
                sl = slabs[s]

                psum_k = ppoolA.tile([128, R, 4, CR], f32, tag="pk")
                psum_q = ppoolA.tile([128, R, CR], f32, tag="pq")
                for r in range(R):
                    for t in range(4):
                        dy, dx = t // 2, t % 2
                        xsl = sl[:, lr + 2 * r + dy, dx::2]
                        nc.tensor.matmul(
                            psum_k[:, r, t, :], xsl, wqk_s[:, 0:CR],
                            start=True, stop=True,
                        )
                        nc.tensor.matmul(
                            psum_q[:, r, :], xsl, wqk_s[:, CR : 2 * CR],
                            start=(t == 0), stop=(t == 3),
                        )

                qs = wpool.tile([128, R, CR], f32, tag="qs")
                nc.scalar.copy(qs[:], psum_q[:])
                prod = wpool.tile([128, R, 4, CR], f32, tag="prod")
                _q = qs[:]
                qs_b = bass.AP(
                    _q.tensor, _q.offset, _q.ap[:2] + [[0, 4]] + _q.ap[2:]
                )
                nc.vector.tensor_tensor(
                    prod[:], psum_k[:], qs_b, mybir.AluOpType.mult
                )
                logit = wpool.tile([128, R, 4], f32, tag="logit")
                nc.vector.tensor_reduce(
                    logit[:], prod[:], mybir.AxisListType.X, mybir.AluOpType.add
                )
                el = wpool.tile([128, R, 4], f32, tag="el")
                nc.scalar.activation(
                    el[:], logit[:], mybir.ActivationFunctionType.Exp
                )
                zsum = wpool.tile([128, R], f32, tag="zsum")
                nc.vector.tensor_reduce(
                    zsum[:], el[:], mybir.AxisListType.X, mybir.AluOpType.add
                )
                rz = wpool.tile([128, R], f32, tag="rz")
                nc.vector.reciprocal(rz[:], zsum[:])
                wgt = wpool.tile([128, R, 4], bf16, tag="wgt", bufs=4)
                _rz = rz[:]
                rz_b = bass.AP(_rz.tensor, _rz.offset, _rz.ap + [[0, 4]])
                nc.vector.tensor_tensor(
                    wgt[:], el[:], rz_b, mybir.AluOpType.mult
                )
                state[("wgt", ch)] = wgt

            def stage_a2(ch):
                """Transpose wgt -> wts [4(t), R, 128(w)] (SBUF bf16)."""
                wgt = state.pop(("wgt", ch))
                psum_wt = ppoolB.tile([4, R, 128], bf16, tag="pwt")
                for r in range(R):
                    nc.tensor.transpose(
                        psum_wt[:, r, :], wgt[:, r, :], ident_s[:]
                    )
                wts = wpool.tile([4, R, 128], bf16, tag="wts", bufs=4)
                nc.scalar.copy(wts[:], psum_wt[:])
                state[("wts", ch)] = wts

            def stage_b(ch):
                """Broadcast weights over channels, weighted window sum."""
                s, lr = CHUNK_SLAB[ch]
                sl = slabs[s]
                wts = state.pop(("wts", ch))

                if ch % 2 == 0:
                    state["acc"] = wpool.tile(
                        [128, 2, R, NW], bf16, tag="acc", name="acc", bufs=2
                    )
                acc = state["acc"]

                ui = wpool.tile([128, R, 2, W], bf16, tag="ui")
                # quarter (h, dy): 2 broadcasts (t=2dy+dx) into a 1-bank psum
                # tile, then one scalar evac into ui[c, r, dy, (m,dx)].
                for h in range(2):
                    wts_h = wts[:, 2 * h : 2 * h + 2, :].rearrange(
                        "k r m -> k (r m)"
                    )  # [4, 256]
                    for dy in (0, 1):
                        pu = ppoolA.tile([128, 2, 2, 128], f32, tag="pu",
                                         bufs=3)
                        for dx in (0, 1):
                            t = 2 * dy + dx
                            nc.tensor.matmul(
                                pu[:, dx], sel_s[:, t, :], wts_h,
                                start=True, stop=True,
                            )
                        p = pu[:]
                        src = bass.AP(
                            p.tensor, p.offset,
                            [p.ap[0], [256, 2], [128, 2], [1, 128]],
                        )
                        u = ui[:]
                        dst = bass.AP(
                            u.tensor,
                            u.offset + (2 * h) * (2 * W) + dy * W,
                            [u.ap[0], [1, 2], [2 * W, 2], [2, 128]],
                        )
                        nc.scalar.copy(dst, src)

                # vv[c, (r,dy), w] = x * ui
                vv = wpool.tile([128, 2 * R, W], bf16, tag="vv")
                ui_flat = ui[:].rearrange("c r d w -> c (r d) w")
                nc.vector.tensor_tensor(
                    vv[:], sl[:, lr : lr + 2 * R, :], ui_flat,
                    mybir.AluOpType.mult,
                )
                vs = wpool.tile([128, R, W], bf16, tag="vs")
                nc.vector.tensor_tensor(
                    vs[:], vv[:, 0 : 2 * R : 2, :], vv[:, 1 : 2 * R : 2, :],
                    mybir.AluOpType.add,
                )
                nc.gpsimd.tensor_tensor(
                    acc[:, ch % 2], vs[:, :, 0::2], vs[:, :, 1::2],
                    mybir.AluOpType.add,
                )
                if ch % 2 == 1:
                    nc.sync.dma_start(
                        out[:, (ch - 1) * R : (ch + 1) * R, :],
                        acc[:].rearrange("p a r w -> p (a r) w"),
                    )

            # 4-deep pipeline: b(ch-3), a2(ch-2), a1(ch)
            for it in range(N_CHUNKS + 3):
                if it < N_CHUNKS:
                    s, _ = CHUNK_SLAB[it]
                    if it == SLAB_FIRST_CHUNK[s] and s + N_HEAD_SLABS < N_SLABS:
                        load_slab(s + N_HEAD_SLABS)
                if 0 <= it - 3:
                    stage_b(it - 3)
                if 0 <= it - 2 < N_CHUNKS:
                    stage_a2(it - 2)
                if it < N_CHUNKS:
                    stage_a1(it)

    nc.compile()
    return nc


_NC_CACHE = {}


def _get_nc():
    if "nc" not in _NC_CACHE:
        _NC_CACHE["nc"] = build_bass()
    return _NC_CACHE["nc"]


def _make_in_maps(fm, Wq, Wk):
    wq_eff = (Wq.astype(np.float64) * (CR ** -0.5) / 4.0).astype(np.float32)
    wqk = np.concatenate([Wk.astype(np.float32), wq_eff], axis=1)
    wqk = np.ascontiguousarray(wqk)
    ident = np.eye(128, dtype=np.float32)
    sel = np.zeros((4, 4, 128), dtype=np.float32)
    for t in range(4):
        sel[t, t, :] = 1.0
    return [
        {
            "fm": np.ascontiguousarray(fm[i]),
            "wqk": wqk,
            "ident": ident,
            "sel": sel,
        }
        for i in range(fm.shape[0])
    ]


def kernel(fm, Wq, Wk):
    from concourse.bass_utils import run_bass_kernel_spmd

    fm = np.asarray(fm, dtype=np.float32)
    Wq = np.asarray(Wq, dtype=np.float32)
    Wk = np.asarray(Wk, dtype=np.float32)

    nc = _get_nc()
    in_maps = _make_in_maps(fm, Wq, Wk)
    res = run_bass_kernel_spmd(nc, in_maps, core_ids=list(range(N_CORES)))
    outs = [np.asarray(res.results[i]["out"]).astype(np.float32) for i in range(N_CORES)]
    return np.stack(outs, axis=0)


# revision 3
# speedup vs baseline: 1.2498x; 1.2498x over previous
"""AttentionDownSample Trainium2 kernel v9 (8 NeuronCores, data-parallel over batch).

vs v5:
  - transpose + sel-broadcast fused into one PE op per (r, t): matmul with
    lhsT = wgt[:, r, t] replicated via a 0-stride free dim (128 identical
    columns) and rhs = identity:
        out[c, m] = sum_w wgt[w] * ident[w, m] = wgt[m]  for all c.
    Removes stage a2 (4 transposes + pwt PSUM + wts scalar copy + sel
    const) and 2 cross-engine hops from the per-chunk chain.
  - pu quarter is now per-r: [128, 4(t), 128(m)] f32 = 1 bank, bufs=4
    (bank budget: pk*2 + pq*2 + pu*4 = 8).
  - 3-stage pipeline: b(ch-2), a1(ch).
"""

import os
import sys

sys.path.insert(0, "/opt/trn_rl_repo")

import numpy as np

B, C, H, W = 8, 128, 256, 256
CR = 32
NH, NW = H // 2, W // 2
N_CORES = 8
R = 4                      # output rows per chunk
N_CHUNKS = NH // R         # 32
SLAB_ROWS_LIST = [8, 24, 32, 32, 32, 32, 32, 32, 32]  # tapered head
N_SLABS = len(SLAB_ROWS_LIST)
SLAB_BASE = [sum(SLAB_ROWS_LIST[:i]) for i in range(N_SLABS)]
CHUNKS_PER_SLAB_LIST = [r // (2 * R) for r in SLAB_ROWS_LIST]
CHUNK_SLAB = []  # chunk -> (slab, local row base)
for _s, _n in enumerate(CHUNKS_PER_SLAB_LIST):
    for _c in range(_n):
        CHUNK_SLAB.append((_s, _c * 2 * R))
SLAB_FIRST_CHUNK = {}
for _ch, (_s, _lr) in enumerate(CHUNK_SLAB):
    if _s not in SLAB_FIRST_CHUNK:
        SLAB_FIRST_CHUNK[_s] = _ch


def build_bass():
    import concourse.bass as bass
    import concourse.mybir as mybir
    from concourse import bacc, tile

    f32 = mybir.dt.float32
    bf16 = mybir.dt.bfloat16
    nc = bacc.Bacc()

    fm = nc.declare_dram_parameter("fm", [C, H, W], f32, isOutput=False)
    wqk = nc.declare_dram_parameter("wqk", [C, 2 * CR], f32, isOutput=False)
    ident = nc.declare_dram_parameter("ident", [128, 128], f32, isOutput=False)
    out = nc.declare_dram_parameter("out", [C, NH, NW], bf16, isOutput=True)

    with tile.TileContext(nc) as tc:
        with (
            tc.tile_pool(name="const", bufs=1) as cpool,
            tc.tile_pool(name="slabs", bufs=1) as spool,
            tc.tile_pool(name="work", bufs=3) as wpool,
            tc.tile_pool(name="psA", bufs=2, space="PSUM") as ppoolA,
        ):
            slabs = [None] * N_SLABS

            def load_slab(s):
                # row-pair/skip split: contiguous 2-row (2KB f32) descriptor
                # runs instead of 1KB single rows -> less per-descriptor
                # overhead on the HBM read side.
                rows = SLAB_ROWS_LIST[s]
                sl = spool.tile([C, rows, W], bf16, tag=f"slab{s}")
                base = SLAB_BASE[s]
                fv = fm[:, base : base + rows, :]
                dv = sl[:]
                for half in (0, 1):
                    src = bass.AP(
                        fv.tensor, fv.offset + half * 2 * W,
                        [fv.ap[0], [4 * W, rows // 4], [1, 2 * W]],
                    )
                    dst = bass.AP(
                        dv.tensor, dv.offset + half * 2 * W,
                        [dv.ap[0], [4 * W, rows // 4], [1, 2 * W]],
                    )
                    nc.gpsimd.dma_start(dst, src)
                slabs[s] = sl

            # slab0 first so SDMA starts streaming immediately; then the
            # consts; then ALL remaining slab descriptor-gens up front so the
            # gpsimd queue never blocks descriptor generation behind compute.
            load_slab(0)
            wqk_s = cpool.tile([C, 2 * CR], bf16)
            nc.gpsimd.dma_start(wqk_s[:], wqk[:])
            ident_s = cpool.tile([128, 128], bf16)
            nc.gpsimd.dma_start(ident_s[:], ident[:])
            for s in range(1, N_SLABS):
                load_slab(s)

            state = {}

            def stage_a1(ch):
                """Projections + softmax chain -> wgt[ch] (SBUF bf16)."""
                s, lr = CHUNK_SLAB[ch]
                sl = slabs[s]

                psum_k = ppoolA.tile([128, R, 4, CR], f32, tag="pk")
                psum_q = ppoolA.tile([128, R, CR], f32, tag="pq")
                for r in range(R):
                    for t in range(4):
                        dy, dx = t // 2, t % 2
                        xsl = sl[:, lr + 2 * r + dy, dx::2]
                        nc.tensor.matmul(
                            psum_k[:, r, t, :], xsl, wqk_s[:, 0:CR],
                            start=True, stop=True,
                        )
                        nc.tensor.matmul(
                            psum_q[:, r, :], xsl, wqk_s[:, CR : 2 * CR],
                            start=(t == 0), stop=(t == 3),
                        )

                qs = wpool.tile([128, R, CR], f32, tag="qs")
                nc.scalar.copy(qs[:], psum_q[:])
                prod = wpool.tile([128, R, 4, CR], f32, tag="prod")
                _q = qs[:]
                qs_b = bass.AP(
                    _q.tensor, _q.offset, _q.ap[:2] + [[0, 4]] + _q.ap[2:]
                )
                nc.vector.tensor_tensor(
                    prod[:], psum_k[:], qs_b, mybir.AluOpType.mult
                )
                logit = wpool.tile([128, R, 4], f32, tag="logit")
                nc.vector.tensor_reduce(
                    logit[:], prod[:], mybir.AxisListType.X, mybir.AluOpType.add
                )
                el = wpool.tile([128, R, 4], f32, tag="el")
                nc.scalar.activation(
                    el[:], logit[:], mybir.ActivationFunctionType.Exp
                )
                zsum = wpool.tile([128, R], f32, tag="zsum")
                nc.vector.tensor_reduce(
                    zsum[:], el[:], mybir.AxisListType.X, mybir.AluOpType.add
                )
                rz = wpool.tile([128, R], f32, tag="rz")
                nc.vector.reciprocal(rz[:], zsum[:])
                wgt = wpool.tile([128, R, 4], bf16, tag="wgt", bufs=6)
                _rz = rz[:]
                rz_b = bass.AP(_rz.tensor, _rz.offset, _rz.ap + [[0, 4]])
                nc.vector.tensor_tensor(
                    wgt[:], el[:], rz_b, mybir.AluOpType.mult
                )
                state[("wgt", ch)] = wgt

            def stage_b(ch):
                """Fused transpose-broadcast + weighted window sum."""
                s, lr = CHUNK_SLAB[ch]
                sl = slabs[s]
                wgt = state.pop(("wgt", ch))

                if ch % 2 == 0:
                    state["acc"] = wpool.tile(
                        [128, 2, R, NW], bf16, tag="acc", name="acc", bufs=3
                    )
                acc = state["acc"]

                ui = wpool.tile([128, R, 2, W], bf16, tag="ui", bufs=4)
                wg = wgt[:]
                for r in range(R):
                    pu = ppoolA.tile([128, 4, 128], f32, tag="pu", bufs=4)
                    for t in range(4):
                        # lhsT = wgt[:, r, t] replicated to 128 columns
                        lhsT = bass.AP(
                            wg.tensor, wg.offset + r * 4 + t,
                            [wg.ap[0], [0, 128]],
                        )
                        nc.tensor.matmul(
                            pu[:, t, :], lhsT, ident_s[:],
                            start=True, stop=True,
                        )
                    # evac: src (dy, dx, m) -> dst ui[c, r, dy, (m,dx)]
                    p = pu[:]
                    src = bass.AP(
                        p.tensor, p.offset,
                        [p.ap[0], [256, 2], [128, 2], [1, 128]],
                    )
                    u = ui[:]
                    dst = bass.AP(
                        u.tensor, u.offset + r * (2 * W),
                        [u.ap[0], [W, 2], [1, 2], [2, 128]],
                    )
                    nc.scalar.copy(dst, src)

                vv = wpool.tile([128, 2 * R, W], bf16, tag="vv", bufs=4)
                ui_flat = ui[:].rearrange("c r d w -> c (r d) w")
                nc.vector.tensor_tensor(
                    vv[:], sl[:, lr : lr + 2 * R, :], ui_flat,
                    mybir.AluOpType.mult,
                )
                vs = wpool.tile([128, R, W], bf16, tag="vs", bufs=4)
                nc.vector.tensor_tensor(
                    vs[:], vv[:, 0 : 2 * R : 2, :], vv[:, 1 : 2 * R : 2, :],
                    mybir.AluOpType.add,
                )
                nc.gpsimd.tensor_tensor(
                    acc[:, ch % 2], vs[:, :, 0::2], vs[:, :, 1::2],
                    mybir.AluOpType.add,
                )
                if ch % 2 == 1:
                    nc.sync.dma_start(
                        out[:, (ch - 1) * R : (ch + 1) * R, :],
                        acc[:].rearrange("p a r w -> p (a r) w"),
                    )

            for it in range(N_CHUNKS + 3):
                if 0 <= it - 3:
                    stage_b(it - 3)
                if it < N_CHUNKS:
                    stage_a1(it)

    nc.compile()
    return nc


_NC_CACHE = {}


def _get_nc():
    if "nc" not in _NC_CACHE:
        _NC_CACHE["nc"] = build_bass()
    return _NC_CACHE["nc"]


def _make_in_maps(fm, Wq, Wk):
    wq_eff = (Wq.astype(np.float64) * (CR ** -0.5) / 4.0).astype(np.float32)
    wqk = np.concatenate([Wk.astype(np.float32), wq_eff], axis=1)
    wqk = np.ascontiguousarray(wqk)
    ident = np.eye(128, dtype=np.float32)
    return [
        {
            "fm": np.ascontiguousarray(fm[i]),
            "wqk": wqk,
            "ident": ident,
        }
        for i in range(fm.shape[0])
    ]


def kernel(fm, Wq, Wk):
    from concourse.bass_utils import run_bass_kernel_spmd

    fm = np.asarray(fm, dtype=np.float32)
    Wq = np.asarray(Wq, dtype=np.float32)
    Wk = np.asarray(Wk, dtype=np.float32)

    nc = _get_nc()
    in_maps = _make_in_maps(fm, Wq, Wk)
    res = run_bass_kernel_spmd(nc, in_maps, core_ids=list(range(N_CORES)))
    outs = [np.asarray(res.results[i]["out"]).astype(np.float32) for i in range(N_CORES)]
    return np.stack(outs, axis=0)
